# revision 47
# baseline (speedup 1.0000x reference)
"""Trainium2 Bass kernel for nn_DiscreteContinuousConv2d (sparse DISCO conv).

Math (see reference):
    xq   = x * quadrature_weights               (B, C, n_in)
    xk   = segment_sum(xq[psi_idx_in] * psi_vals, k*n_out + psi_idx_out)
    out  = einsum("knbc,ock->bon", xk, weight) + bias

Kernel reformulation (Y-form): fold the channel/kernel mixing BEFORE the
sparse contraction.  With
    U[i, k, b, oc] = sum_c x[b, c, i] * weight[oc, c, k]        (dense, on PE)
and val'[e] = psi_vals[e] * qw[psi_idx_in[e]], the output is a pure
gather/segment-sum over the sparse entries:
    out[b, oc, o] = sum_{e in bin o} val'[e] * U[idx_in[e], idx_k[e], b, oc] + bias

Distribution: output-sharded over the 8 cores (core r owns o in
[r*2048, (r+1)*2048)); entries are bucketed per core on the host; no
collectives.  Per core, group-major phases (group g = idx_in>>12, the int16
range of a gather index):
    1. PE builds the U rows of group g (fp16, 36864 rows x 64) -> DRAM.
       Build of group g+1 overlaps with phase-g gathers.  PSUM->SBUF fp16
       casts run on the (otherwise idle) Activation engine.
    2. Per o-tile: dma_gather fetches each entry's U row from the group.
       The gather element is 256B = a PAIR of adjacent U rows (j'//2);
       entries are parity-sorted per (o-tile, group) so a slot's used half
       (A = even row, B = odd row) is a STATIC slice of the pair.  The
       per-tile gathers round-robin 4 SWDGE queues (desc-gen, the dominant
       cost, runs on 4 Q7 core pairs).  Per-bucket slot counts are trimmed
       (A padded to 128, B to 16, trailing pad idx = -1 so the SWDGE ucode
       skips it) instead of one global max.
    3. The segment-sum one-hot, with val' (and the fp16 2^14 scale) folded
       in, is built ON-CHIP by two DVE broadcast ops per tile
       (is_equal(iota, o_loc[slot]) * val[slot]) from a compact 4B/slot
       (o_loc, val) table -- no 54MB one-hot DMA.  PE matmuls (lhsT=one-hot,
       rhs=gathered half) accumulate the 128-bin segment sums in PSUM; the
       16 o-tile accumulators live in SBUF across the 4 phases (DVE adds);
       +bias, *2^-14 unscale; DMA out.

Host-side work is limited to index/layout preprocessing of the sparse
pattern (bucket/sort/pad, fold quadrature weights into per-entry values)
and the final unshard.
"""

import numpy as np
from contextlib import ExitStack

import concourse.bass as bass
import concourse.mybir as mybir
import concourse.tile as tile
from concourse import bacc
from concourse.bass_utils import run_bass_kernel_spmd

P = 128
N_CORES = 8
B, C, OC, K = 2, 32, 32, 9
N_IN = 16384
N_OUT = 16384
O_PER_CORE = N_OUT // N_CORES          # 2048
O_TILES = O_PER_CORE // P              # 16
V64 = B * OC                           # 64 values per U row
KV = K * V64                           # 576
N_GRP = 4                              # idx_in >> 12 gather groups
I_GRP = N_IN // N_GRP                  # 4096
ROWS_GRP = I_GRP * K                   # 36864 U rows per group (18432 pairs)
CH_GRP = I_GRP // P                    # 32 U-build chunks per group
UB = 4                                 # U chunks per DMA write
OH_LOOKAHEAD = 4                       # one-hot build tiles ahead of matmuls
SCALE = np.float32(2.0 ** 14)          # keeps fp16 contributions in normal range
UNIFORM_BUCKETS = True                 # bisect flag: uniform gather sizes

F16 = mybir.dt.float16
F32 = mybir.dt.float32
I16 = mybir.dt.int16


# --------------------------------------------------------------------------
# host-side preprocessing: pure index/layout work on the sparse pattern
# --------------------------------------------------------------------------

def _host_prep(inputs):
    qw = np.asarray(inputs["quadrature_weights"], np.float32)
    vals = np.asarray(inputs["psi_vals"], np.float32)
    ik = np.asarray(inputs["psi_idx_k"]).astype(np.int64)
    io = np.asarray(inputs["psi_idx_out"]).astype(np.int64)
    ii = np.asarray(inputs["psi_idx_in"]).astype(np.int64)

    val2 = (vals * qw[ii] * SCALE).astype(np.float32)
    grp = ii >> 12                                   # gather group 0..3
    jloc = (ii & (I_GRP - 1)) * K + ik               # row within group < 36864
    jpair = (jloc >> 1).astype(np.int64)             # 256B pair index < 18432
    parity = (jloc & 1).astype(np.int64)

    core = io >> 11                                  # owning core
    otile = (io >> 7) & (O_TILES - 1)                # o-tile within core
    o_loc = io & (P - 1)

    # sort all entries by (core, group, o-tile, parity); per-bucket static
    # slot layout: [A slots parity0 (128-pad) | B slots parity1 (16-pad)]
    key = (((core * N_GRP + grp) * O_TILES + otile) * 2 + parity)
    order = np.argsort(key, kind="stable")
    key_s = key[order]
    jp_s, ol_s, v_s = jpair[order], o_loc[order], val2[order]
    n_keys = N_CORES * N_GRP * O_TILES * 2
    bounds = np.searchsorted(key_s, np.arange(n_keys + 1))
    counts = (bounds[1:] - bounds[:-1]).reshape(N_CORES, N_GRP, O_TILES, 2)

    # per-(g,t) sizes, uniform across cores (same compiled program)
    maxA = counts[..., 0].max(axis=0)                # [N_GRP, O_TILES]
    maxB = counts[..., 1].max(axis=0)
    if UNIFORM_BUCKETS:
        maxA = np.full_like(maxA, counts[..., 0].max())
        maxB = np.full_like(maxB, counts[..., 1].max())
        GA = ((maxA + P - 1) // P) * P
        GB = ((maxB + P - 1) // P) * P
    else:
        GA = ((maxA + P - 1) // P) * P
        GB = ((maxB + 15) // 16) * 16
    n16 = GA + GB                                    # gather num_idxs per (g,t)
    acols = GA // P
    nch = (GA + GB + P - 1) // P                     # matmul/one-hot chunks
    assert n16.min() == n16.max(), "uniform bucket sizes required"
    off16 = np.zeros((N_GRP, O_TILES), np.int64)     # jidx col offsets (int16)
    offc = np.zeros((N_GRP, O_TILES), np.int64)      # one-hot chunk offsets
    c16 = c_oh = 0
    for g in range(N_GRP):
        for t in range(O_TILES):
            off16[g, t] = c16
            offc[g, t] = c_oh
            c16 += n16[g, t] // 16
            c_oh += nch[g, t]
    TOT16, TOTC = c16, c_oh

    jidx = np.zeros((N_CORES, P, TOT16), np.int16)
    oloc = np.full((N_CORES, P, TOTC), -1.0, np.float16)
    ovals = np.zeros((N_CORES, P, TOTC), np.float16)
    for r in range(N_CORES):
        for g in range(N_GRP):
            for t in range(O_TILES):
                n = int(n16[g, t])
                ga = int(GA[g, t])
                ncol = int(nch[g, t])
                jp_slot = np.zeros(n, np.int16)
                ol_slot = np.full(ncol * P, -1.0, np.float16)
                v_slot = np.zeros(ncol * P, np.float16)
                kk = (((r * N_GRP + g) * O_TILES + t) * 2)
                loA, hiA = bounds[kk], bounds[kk + 1]
                loB, hiB = bounds[kk + 1], bounds[kk + 2]
                cA, cB = hiA - loA, hiB - loB
                jp_slot[:cA] = jp_s[loA:hiA]
                ol_slot[:cA] = ol_s[loA:hiA]
                v_slot[:cA] = v_s[loA:hiA]
                jp_slot[ga:ga + cB] = jp_s[loB:hiB]
                ol_slot[ga:ga + cB] = ol_s[loB:hiB]
                v_slot[ga:ga + cB] = v_s[loB:hiB]
                if not UNIFORM_BUCKETS:
                    jp_slot[ga + cB:] = -1            # trailing: ucode skips
                # idx wrap [16, n/16], replicated to the 8 16-partition groups
                o16 = int(off16[g, t])
                jidx[r, :, o16:o16 + n // 16] = np.tile(
                    jp_slot.reshape(n // 16, 16).T, (8, 1))
                # one-hot metadata: slot s -> (partition s%128, col s//128)
                oc0 = int(offc[g, t])
                oloc[r, :, oc0:oc0 + ncol] = ol_slot.reshape(ncol, P).T
                ovals[r, :, oc0:oc0 + ncol] = v_slot.reshape(ncol, P).T

    weight = np.asarray(inputs["weight"], np.float32)      # (OC, C, K)
    w16 = weight.transpose(1, 2, 0).reshape(C, K * OC).astype(np.float16)
    w16 = np.ascontiguousarray(np.concatenate([w16, w16], axis=0))  # (64, 288)

    bias = np.asarray(inputs["bias"], np.float32)
    bias_t = np.ascontiguousarray(
        np.broadcast_to(np.tile(bias, B)[None, :], (P, V64))).astype(np.float32)

    # iota_big[p, o, c] = o  (contiguous in0 for the one-hot is_equal; the
    # broadcast operands sit on the middle dim, which DVE streams at full rate)
    ncmax = int(nch.max())
    iota = np.ascontiguousarray(np.broadcast_to(
        np.arange(P, dtype=np.float16)[None, :, None],
        (P, P, ncmax)).reshape(P, P * ncmax))

    x = np.ascontiguousarray(np.asarray(inputs["x"], np.float32))
    common = dict(x=x, wt=w16, biasrow=bias_t, iota=iota)
    percore = [dict(jidx=np.ascontiguousarray(jidx[r]),
                    oloc=np.ascontiguousarray(oloc[r]),
                    ovals=np.ascontiguousarray(ovals[r]))
               for r in range(N_CORES)]
    meta = dict(GA=GA, GB=GB, n16=n16, acols=acols, nch=nch,
                off16=off16, offc=offc, TOT16=TOT16, TOTC=TOTC,
                NCMAX=int(nch.max()))
    return percore, common, meta


# --------------------------------------------------------------------------
# device program
# --------------------------------------------------------------------------

def _build(meta):
    n16, acols, nch = meta["n16"], meta["acols"], meta["nch"]
    off16, offc = meta["off16"], meta["offc"]
    TOT16, TOTC, NCMAX = meta["TOT16"], meta["TOTC"], meta["NCMAX"]

    nc = bacc.Bacc("TRN2", target_bir_lowering=False, num_swdge_queues=4)

    x_d = nc.dram_tensor("x", [B, C, N_IN], F32, kind="ExternalInput")
    w_d = nc.dram_tensor("wt", [2 * C, K * OC], F16, kind="ExternalInput")
    bias_d = nc.dram_tensor("biasrow", [P, V64], F32, kind="ExternalInput")
    iota_d = nc.dram_tensor("iota", [P, P * NCMAX], F16, kind="ExternalInput")
    j_d = nc.dram_tensor("jidx", [P, TOT16], I16, kind="ExternalInput")
    oloc_d = nc.dram_tensor("oloc", [P, TOTC], F16, kind="ExternalInput")
    oval_d = nc.dram_tensor("ovals", [P, TOTC], F16, kind="ExternalInput")
    # one DRAM U tensor per group: phase-g gathers carry no dependency edges
    # against the interleaved group-(g+1) U writes
    u_ds = [nc.dram_tensor(f"U{g}", [ROWS_GRP, V64], F16, kind="Internal")
            for g in range(N_GRP)]
    out_d = nc.dram_tensor("out", [O_PER_CORE, V64], F32, kind="ExternalOutput")

    with tile.TileContext(nc) as tc, ExitStack() as ctx:
        cpool = ctx.enter_context(tc.tile_pool(name="const", bufs=1))
        # per-group x slices so group-0 U build starts after 1/4 of the load
        xg = []
        for g in range(N_GRP):
            xt = cpool.tile([2 * C, I_GRP], F16, tag=f"x{g}")
            nc.gpsimd.dma_start(
                out=xt[:],
                in_=x_d[:, :, g * I_GRP:(g + 1) * I_GRP].rearrange(
                    "b c n -> (b c) n"))
            xg.append(xt)
        w16 = cpool.tile([2 * C, K * OC], F16)
        nc.sync.dma_start(out=w16[:], in_=w_d[:])
        jall = cpool.tile([P, TOT16], I16)
        nc.sync.dma_start(out=jall[:], in_=j_d[:])
        iota_t = cpool.tile([P, P, NCMAX], F16)
        nc.scalar.dma_start(
            out=iota_t[:], in_=iota_d[:].rearrange("p (o c) -> p o c", c=NCMAX))
        oloc_t = cpool.tile([P, TOTC], F16)
        nc.scalar.dma_start(out=oloc_t[:], in_=oloc_d[:])
        oval_t = cpool.tile([P, TOTC], F16)
        nc.scalar.dma_start(out=oval_t[:], in_=oval_d[:])
        # staged through a DVE copy so downstream DVE ops read same-engine data
        bias_t0 = cpool.tile([P, V64], F32)
        nc.sync.dma_start(out=bias_t0[:], in_=bias_d[:])
        bias_t = cpool.tile([P, V64], F32)
        nc.vector.tensor_copy(out=bias_t[:], in_=bias_t0[:])

        upool = ctx.enter_context(tc.tile_pool(name="usb", bufs=3))
        ypsum = ctx.enter_context(tc.tile_pool(name="ypsum", bufs=4, space="PSUM"))
        gpool = ctx.enter_context(tc.tile_pool(name="gath", bufs=10))
        opool = ctx.enter_context(tc.tile_pool(name="ohv", bufs=OH_LOOKAHEAD + 2))
        opsum = ctx.enter_context(tc.tile_pool(name="opsum", bufs=4, space="PSUM"))
        rpool = ctx.enter_context(tc.tile_pool(name="res", bufs=2))

        # per-group U views: [8 write-blocks, 128 part, (4 chunks x 9 k x 64 v)]
        u_blks = [u.rearrange("(blk c p k) v -> blk p c (k v)", c=UB, p=P, k=K)
                  for u in (u[:] for u in u_ds)]
        u_pairs = [u[:].rearrange("(q two) v -> q (two v)", two=2)  # (18432, 128)
                   for u in u_ds]

        N_UNI = int(n16[0, 0])
        GA_U, GB_U = int(meta["GA"][0, 0]), int(meta["GB"][0, 0])

        # all 16 o-tile accumulators live in SBUF across the 4 phases (PSUM
        # accumulation groups conflict at 2KB zero-region granularity, so a
        # per-tile group is closed within each phase and added here by DVE)
        acc = cpool.tile([P, O_TILES * V64], F32)

        def u_build_block(g, cb):
            # U[(i k), (b oc)] = sum_c x16[(b,c), i] w16[(b,c), (k,oc)]
            u_sb = upool.tile([P, UB * KV], F16)
            u_v = u_sb[:].rearrange(
                "p (c k b2 oc) -> p c k b2 oc", c=UB, k=K, b2=B)
            for c4 in range(UB):
                ch = cb * UB + c4
                for b in range(B):
                    yp = ypsum.tile([P, K * OC], F32)
                    nc.tensor.matmul(
                        out=yp[:],
                        lhsT=xg[g][b * C:(b + 1) * C, ch * P:(ch + 1) * P],
                        rhs=w16[b * C:(b + 1) * C, :],
                        start=True, stop=True)
                    # fp32 PSUM -> fp16 SBUF cast on the Activation engine
                    nc.scalar.activation(
                        out=u_v[:, c4, :, b, :],
                        in_=yp[:].rearrange("p (k oc) -> p k oc", k=K),
                        func=mybir.ActivationFunctionType.Copy)
            nc.sync.dma_start(
                out=u_blks[g][cb],
                in_=u_sb[:].rearrange("p (c kv) -> p c kv", c=UB))

        def oh_build(gi):
            # one-hot lhsT for tile gi, layout [slot_p, o, chunk]:
            # oh[p, o, c] = val[c*128+p] * (o == o_loc[c*128+p]).  in0 (iota)
            # is contiguous and the (oloc, val) broadcasts ride the middle
            # dim, so both DVE ops run at full 16-bit rate.
            g, t = gi // O_TILES, gi % O_TILES
            oc0 = int(offc[g, t])
            oht = opool.tile([P, P, NCMAX], F16, tag="oht")
            oloc_b = oloc_t[:, oc0:oc0 + NCMAX].unsqueeze(1).broadcast_to(
                [P, P, NCMAX])
            oval_b = oval_t[:, oc0:oc0 + NCMAX].unsqueeze(1).broadcast_to(
                [P, P, NCMAX])
            nc.vector.tensor_tensor(
                out=oht[:], in0=iota_t[:], in1=oloc_b,
                op=mybir.AluOpType.is_equal)
            nc.vector.tensor_tensor(
                out=oht[:], in0=oht[:], in1=oval_b, op=mybir.AluOpType.mult)
            return oht

        def issue_gather(gi, gt):
            g, t = gi // O_TILES, gi % O_TILES
            nc.gpsimd.dma_gather(
                gt,
                u_pairs[g],
                jall[:, int(off16[g, t]):int(off16[g, t]) + N_UNI // 16],
                N_UNI, N_UNI, 2 * V64, elem_step=2 * V64, single_packet=False,
                queue_num=gi % 4)

        def consume_tile(gi, gt, oht):
            g, t = gi // O_TILES, gi % O_TILES
            ncol = int(nch[g, t])
            ac = int(acols[g, t])
            ps = opsum.tile([P, V64], F32)
            for ci in range(ncol):
                half = slice(0, V64) if ci < ac else slice(V64, 2 * V64)
                nc.tensor.matmul(
                    out=ps[:], lhsT=oht[:, :, ci], rhs=gt[:, ci, half],
                    start=(ci == 0), stop=(ci == ncol - 1))
            acct = acc[:, t * V64:(t + 1) * V64]
            if g == 0:
                nc.vector.tensor_copy(out=acct, in_=ps[:])
            else:
                nc.vector.tensor_add(out=acct, in0=acct, in1=ps[:])
            if g == N_GRP - 1:
                res = rpool.tile([P, V64], F32, tag="res")
                nc.vector.scalar_tensor_tensor(
                    out=res[:], in0=acct, scalar=float(1.0 / SCALE),
                    in1=bias_t[:],
                    op0=mybir.AluOpType.mult, op1=mybir.AluOpType.add)
                nc.sync.dma_start(out=out_d[t * P:(t + 1) * P, :], in_=res[:])

        # group-0 U build as prologue; group g+1 interleaves into phase g
        for cb in range(CH_GRP // UB):
            u_build_block(0, cb)
        ohts = {gi: oh_build(gi) for gi in range(OH_LOOKAHEAD)}

        NT = N_GRP * O_TILES
        for gi in range(NT):
            g, t = gi // O_TILES, gi % O_TILES
            if g < N_GRP - 1 and t < CH_GRP // UB:
                u_build_block(g + 1, t)
            if gi + OH_LOOKAHEAD < NT:
                ohts[gi + OH_LOOKAHEAD] = oh_build(gi + OH_LOOKAHEAD)
            gt = gpool.tile([P, NCMAX, 2 * V64], F16, tag="g")
            issue_gather(gi, gt[:])
            consume_tile(gi, gt[:], ohts.pop(gi))

    nc.compile()
    return nc


_last_result = None


def kernel(**inputs) -> np.ndarray:
    global _last_result
    per_core, common, meta = _host_prep(inputs)
    nc = _build(meta)
    in_maps = [{**common, **pc} for pc in per_core]
    r = run_bass_kernel_spmd(nc, in_maps, core_ids=list(range(N_CORES)))
    _last_result = r
    out = np.concatenate([res["out"] for res in r.results], axis=0)  # (16384, 64)
    return np.ascontiguousarray(out.reshape(N_OUT, B, OC).transpose(1, 2, 0))


if __name__ == "__main__":
    rng = np.random.default_rng(0)
    NNZ = 1_500_000
    ins = dict(
        x=rng.standard_normal((B, C, N_IN)).astype(np.float32),
        quadrature_weights=(rng.uniform(0.5, 1.5, N_IN) / N_IN).astype(np.float32),
        psi_vals=rng.uniform(0, 1, NNZ).astype(np.float32),
        weight=(rng.standard_normal((OC, C, K)) / np.sqrt(C)).astype(np.float32),
        bias=np.zeros(OC, np.float32),
        psi_idx_k=rng.integers(0, K, NNZ).astype(np.int32),
        psi_idx_out=rng.integers(0, N_OUT, NNZ).astype(np.int32),
        psi_idx_in=rng.integers(0, N_IN, NNZ).astype(np.int32),
        n_out=N_OUT,
    )
    out = kernel(**ins)
    print("kernel out", out.shape, out.dtype, float(np.abs(out).mean()))
